# revision 39
# baseline (speedup 1.0000x reference)
"""Trainium2 Bass kernel for nn_LorentzLayer.

Math: the reference applies a per-cluster weighted Lorentz boost to T[b,c,:],
sums over clusters, then applies a second (inner) boost:

    out[b,a] = sum_{c,d} (B_inner @ (W_c * B_outer_c))[a,d] * T[b,c,d]

Both boosts compose into a single tiny matrix Mfull (400, 4) applied to
T flattened to (262144, 400):  out = Tf @ Mfull.

Device strategy (8 cores, pure batch data-parallel):
  - Host computes Mfull in float64 (it only depends on the tiny inputs).
  - Accuracy budget is rel-l2 < 2e-2. The kernel is DMA-bound, so input
    bytes are the lever:
      * single-plane bf16 instead of the exact hi+lo fp32 split (2x);
      * the 128 K-rows with the smallest ||Mfull[j,:]|| are shipped as
        fp8 e4m3 (another 4.2 MB/core saved). Row permutation is free:
        host reorders rows of T and Mfull together. Measured rel-l2 err
        ~9.3e-3 on the reference inputs (2.2x under the gate).
  - Host pre-transposes each core's shard to (400, 32768); bf16 K rows
    are packed into `big` (128, 2*B_core) whose per-subtile block
    [chunk0|chunk1] is one contiguous (128, 2*NB) DMA; fp8 rows into
    `big8` (128, B_core); the 16 ragged bf16 rows are folded 4x across
    partitions {0,32,64,96} and loaded once per pass.
  - Per 512-col PSUM tile: 4 matmuls (rag K=16 bf16, 2x K=128 bf16,
    1x K=128 fp8 with hi-only fp8 stationary) accumulate into a
    (4, 512) PSUM group; DVE copies to SBUF; output stored (4, B_core)
    f32 with the store base partition rotating across SDMA engines.
  - Input DMAs alternate between the two HWDGE rings (SP + ACT) as
    whole 128-partition transfers; 64-row half splits measured ~60%
    slower (each covers only half the SDMA engines).
"""

import numpy as np
import ml_dtypes

BF16 = ml_dtypes.bfloat16
F8 = ml_dtypes.float8_e4m3   # TRN fp8e4 flavor (max 240)

BATCH = 262144
CLUSTER = 100
KDIM = 4 * CLUSTER  # 400
NCORES = 8
B_CORE = BATCH // NCORES  # 32768
NB = 2048    # batch subtile (columns per DMA)
NPS = 512    # psum tile free size
RAG = 16     # ragged bf16 rows (KDIM - 3*128)

# Tunables (module-level so probes can flip them)
N8CH = 1            # number of 128-row fp8 chunks (0..2)
BIG_SPLIT = "whole"  # 'whole' | 'half'
OUT_BF16 = False    # store output as bf16
OUT_SWDGE = False   # out stores via SWDGE ring
OUT_ROTATE = True   # rotate out/psum base partition across SDMA engines
RAG_PRELOAD = True  # load all rag data once per pass (folded on partitions)
BUFS_IN = 4
BUFS_PS = 8
BUFS_OUT = 4


def _build_nc(b_core: int, nb: int, repeat: int = 1, mode: str = "full",
              n8ch: int = None, bufs_in: int = None, bufs_ps: int = None,
              bufs_out: int = None, out_bf16: bool = None,
              out_rotate: bool = None, rag_preload: bool = None,
              out_swdge: bool = None, big_split: str = None,
              skip_rag: bool = False, skip_out: bool = False):
    """mode: 'full' | 'dma' (loads only) | 'compute' (no big loads).
    repeat>1 wraps the pass in a device-side For_i loop (timing harness)."""
    import concourse.bacc as bacc
    import concourse.tile as tile
    import concourse.mybir as mybir

    if n8ch is None:
        n8ch = N8CH
    if bufs_in is None:
        bufs_in = BUFS_IN
    if bufs_ps is None:
        bufs_ps = BUFS_PS
    if bufs_out is None:
        bufs_out = BUFS_OUT
    if out_bf16 is None:
        out_bf16 = OUT_BF16
    if out_rotate is None:
        out_rotate = OUT_ROTATE
    if rag_preload is None:
        rag_preload = RAG_PRELOAD
    if out_swdge is None:
        out_swdge = OUT_SWDGE
    if big_split is None:
        big_split = BIG_SPLIT

    nch16 = 3 - n8ch  # 128-row bf16 chunks

    bf16 = mybir.dt.bfloat16
    f8 = mybir.dt.float8e4
    f32 = mybir.dt.float32
    out_dt = bf16 if out_bf16 else f32

    nc = bacc.Bacc("TRN2", target_bir_lowering=False, debug=False,
                   num_devices=NCORES)

    # fused input: per subtile, per partition row the bytes are
    # [chunk0 bf16 | .. | chunk_{nch16-1} bf16 | fp8 chunks], declared as a
    # bf16 tensor of (6-n8ch)*nb/2 cols per subtile; the fp8 region is
    # reached through AP.bitcast so one uniform (128, 10KB-row) DMA feeds
    # all chunks.
    bcols = (6 - n8ch) * nb // 2                # bf16 cols per subtile
    big = nc.dram_tensor("big", (128, bcols * (b_core // nb)), bf16,
                         kind="ExternalInput")
    if rag_preload:
        # rag rows folded 4x across partitions: ragf[16g+r, w] =
        # rag_row[r, g*(b_core//4) + w]; loaded once per pass
        rag = nc.dram_tensor("rag", (4 * RAG, b_core // 4), bf16,
                             kind="ExternalInput")
    else:
        rag = nc.dram_tensor("rag", (RAG, b_core), bf16,
                             kind="ExternalInput")
    stat = nc.dram_tensor("stat", (128, 16), bf16, kind="ExternalInput")
    stat8 = None
    if n8ch:
        stat8 = nc.dram_tensor("stat8", (128, 4 * n8ch), f8,
                               kind="ExternalInput")
    outT = nc.dram_tensor("outT", (4, b_core), out_dt, kind="ExternalOutput")

    n_sub = b_core // nb
    n_ps = nb // NPS
    gcols = b_core // 4
    assert not rag_preload or gcols % NPS == 0
    do_dma = mode in ("full", "dma")
    do_compute = mode in ("full", "compute")

    with tile.TileContext(nc) as tc:
        with (
            tc.tile_pool(name="statp", bufs=1) as statpool,
            tc.tile_pool(name="inp", bufs=bufs_in) as inpool,
            tc.tile_pool(name="ragp", bufs=2) as ragpool,
            tc.tile_pool(name="outp", bufs=bufs_out) as outpool,
            tc.tile_pool(name="ps", bufs=bufs_ps, space="PSUM") as pspool,
        ):
            stat_sb = statpool.tile([128, 16], bf16)
            nc.sync.dma_start(out=stat_sb[:, :], in_=stat[:, :])
            stat8_sb = None
            if n8ch:
                stat8_sb = statpool.tile([128, 4 * n8ch], f8)
                nc.scalar.dma_start(out=stat8_sb[:, :], in_=stat8[:, :])

            if not do_dma:
                dummy_in = statpool.tile([128, bcols], bf16)
                nc.gpsimd.memset(dummy_in[:, :], 0)
                dummy_rag = statpool.tile(
                    [128, gcols if rag_preload else nb], bf16)
                nc.gpsimd.memset(dummy_rag[:, :], 0)

            def aux_eng(s):
                return nc.scalar if s % 2 == 0 else nc.sync

            def pass_body():
                prt = None
                if rag_preload:
                    if do_dma:
                        prt = ragpool.tile([128, gcols], bf16)
                        if not skip_rag:
                            for g in range(4):
                                eng = nc.sync if g % 2 == 0 else nc.scalar
                                eng.dma_start(
                                    out=prt[32 * g:32 * g + RAG, :],
                                    in_=rag[RAG * g:RAG * (g + 1), :])
                    else:
                        prt = dummy_rag
                for s in range(n_sub):
                    q = (0, 64, 32, 96)[s % 4]
                    if rag_preload:
                        rt = prt
                    elif do_dma:
                        rt = ragpool.tile([128, nb], bf16)
                        if not skip_rag:
                            aux_eng(s).dma_start(
                                out=rt[q:q + RAG, :],
                                in_=rag[:, s * nb:(s + 1) * nb])
                    else:
                        rt = dummy_rag
                    beng = nc.sync if s % 2 == 0 else nc.scalar
                    oeng = nc.scalar if s % 2 == 0 else nc.sync
                    if do_dma:
                        bt = inpool.tile([128, bcols], bf16, tag="b16")
                        csl = slice(s * bcols, (s + 1) * bcols)
                        if big_split == "whole":
                            beng.dma_start(out=bt[:, :], in_=big[:, csl])
                        elif big_split == "half":
                            nc.sync.dma_start(out=bt[0:64, :],
                                              in_=big[0:64, csl])
                            nc.scalar.dma_start(out=bt[64:128, :],
                                                in_=big[64:128, csl])
                        else:
                            raise ValueError(big_split)
                    else:
                        bt = dummy_in
                    # out/psum base partition rotates (tile_position only
                    # allows multiples of 32) so the (4, nb) store doesn't
                    # pile onto the one engine serving partitions 0-7
                    q2 = (0, 64, 32, 96)[(s + 2) % 4] if out_rotate else 0
                    ot = outpool.tile([128, nb] if out_rotate else [4, nb],
                                      out_dt)
                    ots = ot[q2:q2 + 4, :]
                    if not do_compute:
                        nc.gpsimd.memset(ots[:, 0:1], 0)
                    if do_compute:
                        for j in range(n_ps):
                            ps = pspool.tile([128, NPS] if out_rotate
                                             else [4, NPS], f32)
                            pss = ps[q2:q2 + 4, :]
                            jsl = slice(j * NPS, (j + 1) * NPS)
                            do_rag = not (skip_rag and do_dma)
                            if do_rag and rag_preload:
                                c0 = s * nb + j * NPS
                                g, w0 = c0 // gcols, c0 % gcols
                                # PSUM start=True clears the whole bank
                                nc.tensor.matmul(
                                    pss[:, :],
                                    stat_sb[32 * g:32 * g + RAG, 12:16],
                                    rt[32 * g:32 * g + RAG, w0:w0 + NPS],
                                    start=True, stop=False,
                                    tile_position=(32 * g, q2))
                            elif do_rag:
                                nc.tensor.matmul(pss[:, :],
                                                 stat_sb[q:q + RAG, 12:16],
                                                 rt[q:q + RAG, jsl],
                                                 start=True, stop=False,
                                                 tile_position=(q, q2))
                            for k in range(nch16):
                                ksl = slice(k * nb + j * NPS,
                                            k * nb + (j + 1) * NPS)
                                nc.tensor.matmul(
                                    pss[:, :],
                                    stat_sb[:, 4 * k:4 * k + 4],
                                    bt[:, ksl],
                                    start=(k == 0 and not do_rag),
                                    stop=(k == nch16 - 1 and not n8ch),
                                    tile_position=(0, q2))
                            for k in range(n8ch):
                                off = nch16 * nb + k * nb // 2
                                ksl = slice(off + j * NPS // 2,
                                            off + (j + 1) * NPS // 2)
                                nc.tensor.matmul(
                                    pss[:, :],
                                    stat8_sb[:, 4 * k:4 * k + 4],
                                    bt[:, ksl].bitcast(f8),
                                    start=False, stop=(k == n8ch - 1),
                                    tile_position=(0, q2))
                            nc.vector.tensor_copy(ots[:, jsl], pss[:, :])
                    if do_dma and not skip_out:
                        seng = nc.gpsimd if out_swdge else oeng
                        seng.dma_start(
                            out=outT[:, s * nb:(s + 1) * nb], in_=ots[:, :])

            if repeat > 1:
                with tc.For_i(0, repeat, 1,
                              hint_engines=(mybir.EngineType.PE,
                                            mybir.EngineType.DVE,
                                            mybir.EngineType.SP,
                                            mybir.EngineType.Activation)):
                    pass_body()
            else:
                pass_body()

    nc.compile()
    return nc


def _boost_mats(boosts: np.ndarray, K_mats: np.ndarray) -> np.ndarray:
    """boosts (C,3) -> Lorentz boost matrices (C,4,4), float64."""
    b = boosts.astype(np.float64)
    K = K_mats.astype(np.float64)
    mag = np.sqrt((b * b).sum(axis=1, keepdims=True))        # (C,1)
    n = b / mag                                              # (C,3)
    g = 1.0 / np.sqrt(1.0 - mag * mag)                       # (C,1)
    nK = np.einsum('cj,jad->cad', n, K)                      # (C,4,4)
    nK2 = np.einsum('cab,cbd->cad', nK, nK)                  # (C,4,4)
    B = (np.eye(4)[None]
         - (g * mag)[..., None] * nK
         + (g - 1.0)[..., None] * nK2)
    return B


def _mfull(Bo, Bi, W, K_mats) -> np.ndarray:
    """Composite matrix Mfull (400, 4): out[b,a] = sum_j Tf[b,j] Mfull[j,a]."""
    Bc = _boost_mats(Bo, K_mats)                  # (C,4,4)
    B2 = _boost_mats(Bi, K_mats)[0]               # (4,4)
    comp = np.einsum('ad,cde->cae', B2, Bc)       # (C,4,4) = B2 @ Bc
    comp = comp * W.astype(np.float64)[:, None]   # weight per cluster
    # Mfull[c*4+d, a] = comp[c, a, d]
    return np.ascontiguousarray(comp.transpose(0, 2, 1).reshape(KDIM, 4))


def _row_split(Mfull64: np.ndarray, n8ch: int):
    """Row assignment: the 128*n8ch smallest-||M|| rows go fp8; of the
    rest, the first 128*(3-n8ch) go to bf16 chunks, the last 16 to rag."""
    order = np.argsort(np.linalg.norm(Mfull64, axis=1), kind="stable")
    idx8 = order[:128 * n8ch]
    rest = np.sort(order[128 * n8ch:])
    return idx8, rest[:128 * (3 - n8ch)], rest[128 * (3 - n8ch):]


def _pack_stationary(Mfull64: np.ndarray, n8ch: int):
    """-> stat (128, 16) bf16, stat8 (128, 4*n8ch) fp8."""
    idx8, idx16, idxrag = _row_split(Mfull64, n8ch)
    Mb = Mfull64.astype(np.float32).astype(BF16)  # (400, 4)
    stat = np.zeros((128, 16), dtype=BF16)
    for k in range(3 - n8ch):
        stat[:, 4 * k:4 * k + 4] = Mb[idx16[k * 128:(k + 1) * 128]]
    for qi in range(4):
        stat[32 * qi:32 * qi + RAG, 12:16] = Mb[idxrag]
    stat8 = np.zeros((128, max(4 * n8ch, 4)), dtype=F8)
    if n8ch:
        M8 = Mfull64.astype(np.float32).astype(F8)
        for k in range(n8ch):
            stat8[:, 4 * k:4 * k + 4] = M8[idx8[k * 128:(k + 1) * 128]]
    return stat, stat8


def _pack_core(Tt: np.ndarray, Mfull64: np.ndarray, b_core: int, nb: int,
               n8ch: int = None, rag_preload: bool = None):
    """Tt (400, b_core) f32 -> {'big','big8','rag'} device layouts."""
    if n8ch is None:
        n8ch = N8CH
    if rag_preload is None:
        rag_preload = RAG_PRELOAD
    idx8, idx16, idxrag = _row_split(Mfull64, n8ch)
    nch16 = 3 - n8ch
    n_sub = b_core // nb
    out = {}
    b16 = Tt[idx16].astype(BF16)                 # (nch16*128, b_core)
    # (128, n_sub, nch16, nb) -> per-subtile bf16 chunk bytes
    p16 = np.ascontiguousarray(
        b16.reshape(nch16, 128, n_sub, nb).transpose(1, 2, 0, 3))
    pieces = [p16.view(np.uint8).reshape(128, n_sub, nch16 * nb * 2)]
    if n8ch:
        b8 = Tt[idx8].astype(F8)                 # (n8ch*128, b_core)
        p8 = np.ascontiguousarray(
            b8.reshape(n8ch, 128, n_sub, nb).transpose(1, 2, 0, 3))
        pieces.append(p8.view(np.uint8).reshape(128, n_sub, n8ch * nb))
    fused = np.concatenate(pieces, axis=2)       # (128, n_sub, (6-n8ch)*nb)
    out["big"] = np.ascontiguousarray(fused).view(BF16).reshape(128, -1)
    ragT = Tt[idxrag].astype(BF16)               # (16, b_core)
    if rag_preload:
        out["rag"] = np.ascontiguousarray(
            ragT.reshape(RAG, 4, b_core // 4).transpose(1, 0, 2)
        ).reshape(4 * RAG, b_core // 4)
    else:
        out["rag"] = np.ascontiguousarray(ragT)
    return out


_NC_CACHE = {}


def _get_nc():
    key = (B_CORE, NB, N8CH, BIG_SPLIT, OUT_BF16, OUT_ROTATE,
           RAG_PRELOAD, OUT_SWDGE, BUFS_IN, BUFS_PS, BUFS_OUT)
    if key not in _NC_CACHE:
        _NC_CACHE[key] = _build_nc(B_CORE, NB)
    return _NC_CACHE[key]


def _selftest_small():
    """CoreSim structural/numeric check at reduced size (no hardware)."""
    from concourse.bass_interp import CoreSim
    b_core_t, nb_t = 2048, 512
    rng = np.random.default_rng(0)
    Tt = rng.standard_normal((KDIM, b_core_t)).astype(np.float32)
    Mfull = rng.standard_normal((KDIM, 4)).astype(np.float64) * 0.3
    stat, stat8 = _pack_stationary(Mfull, N8CH)
    packs = _pack_core(Tt, Mfull, b_core_t, nb_t)
    nc = _build_nc(b_core_t, nb_t)
    sim = CoreSim(nc, require_finite=True, require_nnan=True)
    sim.tensor("stat")[:] = stat
    if N8CH:
        sim.tensor("stat8")[:] = stat8[:, :4 * N8CH]
    for k, v in packs.items():
        sim.tensor(k)[:] = v
    sim.simulate(check_with_hw=False)
    got = np.asarray(sim.tensor("outT"), dtype=np.float32).T  # (b_core, 4)
    # emulate quantization for the expected value
    idx8, idx16, idxrag = _row_split(Mfull, N8CH)
    q = Tt.astype(BF16).astype(np.float64)
    Mq = Mfull.astype(np.float32).astype(BF16).astype(np.float64)
    if N8CH:
        q[idx8] = Tt[idx8].astype(F8).astype(np.float64)
        Mq[idx8] = Mfull[idx8].astype(np.float32).astype(F8).astype(np.float64)
    want = q.T @ Mq
    rel = np.linalg.norm(got - want) / np.linalg.norm(want)
    assert rel < 1e-4, rel
    return rel


def prepare_in_maps(T, Bo, Bi, W, K_mats, nb=None):
    nbv = nb if nb is not None else NB
    T = np.asarray(T, dtype=np.float32)
    Mfull = _mfull(np.asarray(Bo), np.asarray(Bi),
                   np.asarray(W), np.asarray(K_mats))
    stat, stat8 = _pack_stationary(Mfull, N8CH)
    Tf = T.reshape(BATCH, KDIM)
    in_maps = []
    for c in range(NCORES):
        Tt = np.ascontiguousarray(Tf[c * B_CORE:(c + 1) * B_CORE].T)
        m = _pack_core(Tt, Mfull, B_CORE, nbv)
        m["stat"] = stat
        if N8CH:
            m["stat8"] = stat8[:, :4 * N8CH]
        in_maps.append(m)
    return in_maps


# Set by test harnesses to profile the run; kernel() stores the spmd results
# object (exec_time_ns etc.) in LAST_RESULTS when TRACE is on.
TRACE = False
TRACE_KWARGS = {}
LAST_RESULTS = None


def kernel(T, Bo, Bi, W, K_mats):
    from concourse.bass_utils import run_bass_kernel_spmd

    in_maps = prepare_in_maps(T, Bo, Bi, W, K_mats)
    nc = _get_nc()
    res = run_bass_kernel_spmd(nc, in_maps, core_ids=list(range(NCORES)),
                               trace=TRACE, **TRACE_KWARGS)
    if TRACE:
        global LAST_RESULTS
        LAST_RESULTS = res

    out = np.empty((BATCH, 4), dtype=np.float32)
    for c in range(NCORES):
        o4 = np.asarray(res.results[c]["outT"], dtype=np.float32)  # (4, Bc)
        out[c * B_CORE:(c + 1) * B_CORE] = o4.T
    return out.reshape(BATCH, 1, 4)
